# revision 2
# baseline (speedup 1.0000x reference)
"""GATConv forward on 8 Trainium2 NeuronCores (Bass/Tile), v5.

Destination-node sharding (12544 dst nodes / core, 98 tiles of 128), host
renumbers nodes cyclically per core so the SPMD program is identical.

Phase A: T = rows of [h(128) bf16 head-minor | a_src(4) bf16 | pad] (512 B),
A = rows of [a_dst(4) bf16 | pad] (256 B, local nodes). h = x @ W on PE,
channel order (c,h)-minor so later DVE multiplies hit the 2x packed mode.

Phase B, per PAIR of dst tiles: 4 src-bank dma_gathers of T rows + 1
dma_gather of A rows by edge dst (counts in hoisted registers). Banks are
b0 = local rows [0,B) (self loops + local edges), b1..b3 = equal thirds of
the rest (< 32768 rows each for int16 indices). Per tile: z = a_src+a_dst,
w = exp(leakyrelu(z)) into msg[:,:,128:132]; msg[:,:,0:128] = h*w (single
2x op); scatter via one-hot matmuls accumulating [sum w*h | sum w] in PSUM;
out = relu(mean_h(sum w*h / sum w) + bias).
"""
import sys

sys.path.insert(0, "/opt/trn_rl_repo")
import numpy as np
import ml_dtypes

import concourse.bass as bass
import concourse.mybir as mybir
import concourse.tile as tile
from concourse.bass_utils import run_bass_kernel_spmd
from concourse import bacc

BF16 = ml_dtypes.bfloat16
P = 128
N = 100000
NPAD = 100352          # 784 tiles of 128; 8 cores x 12544
NCORE = 8
B = NPAD // NCORE      # 12544 local nodes per core
TPC = B // P           # 98 tiles per core
NPAIR = TPC // 2       # 49
NBANK = 4
NEG = 0.2
H, C = 4, 32
PAD_SLOT = 200.0
EL = 256               # T row: 256 bf16 = 512B

# bank row ranges: b0 = local rows, b1..b3 equal thirds of the remainder
_rest = NPAD - B
_s1 = B + (_rest + 2) // 3
_s2 = B + 2 * ((_rest + 2) // 3)
BANK_LO = np.array([0, B, _s1, _s2], np.int64)
BANK_HI = np.array([B, _s1, _s2, NPAD], np.int64)
assert all(BANK_HI - BANK_LO < 32768)


def _prep_edges(edge_index):
    """K [TPC,NBANK] shared chunk counts and per-core gather tables.

    T-gather stream order: (pair, bank, tile-in-pair, k).
    A-gather stream order: (pair, tile-in-pair, bank, k); per-tile local
    chunk order is bank-major.
    """
    src0 = edge_index[0].astype(np.int64)
    dst0 = edge_index[1].astype(np.int64)
    loops = np.arange(NPAD, dtype=np.int64)
    src = np.concatenate([src0, loops])
    dst = np.concatenate([dst0, loops])

    per_core = []
    cnts = np.zeros((NCORE, TPC * NBANK), np.int64)
    for c in range(NCORE):
        lo, hi = c * B, (c + 1) * B
        sel = (dst >= lo) & (dst < hi)
        d = dst[sel] - lo
        s = (src[sel] - lo) % NPAD
        t = d >> 7
        sl = d & 127
        bk = np.searchsorted(BANK_HI, s, side="right")
        il = s - BANK_LO[bk]
        g = t * NBANK + bk
        order = np.argsort(g, kind="stable")
        per_core.append((g[order], il[order], sl[order], (t * P + sl)[order]))
        cnts[c] = np.bincount(g, minlength=TPC * NBANK)

    K = np.ceil(cnts.max(axis=0) / P).astype(np.int64).reshape(TPC, NBANK)
    TOTC = int(K.sum())

    gbase = np.zeros((TPC, NBANK), np.int64)
    run = 0
    for p in range(NPAIR):
        for b in range(NBANK):
            for ts in range(2):
                gbase[2 * p + ts, b] = run
                run += K[2 * p + ts, b]
    assert run == TOTC
    abase = np.zeros((TPC, NBANK), np.int64)
    run = 0
    for t in range(TPC):
        for b in range(NBANK):
            abase[t, b] = run
            run += K[t, b]
    assert run == TOTC

    idx_maps, slot_maps, dst_maps = [], [], []
    for c in range(NCORE):
        gs, ils, sls, drs = per_core[c]
        cnt = cnts[c]
        start = np.zeros(TPC * NBANK + 1, np.int64)
        np.cumsum(cnt, out=start[1:])
        rank = np.arange(len(gs)) - start[gs]
        t_of = gs // NBANK
        b_of = gs % NBANK
        tpos = gbase[t_of, b_of] * P + rank
        apos = abase[t_of, b_of] * P + rank

        tstream = np.zeros(TOTC * P, np.int16)
        tstream[tpos] = ils.astype(np.int16)
        sstream = np.full(TOTC * P, PAD_SLOT, np.float32)
        sstream[tpos] = sls.astype(np.float32)
        astream = np.zeros(TOTC * P, np.int16)
        astream[apos] = drs.astype(np.int16)

        idx128 = np.ascontiguousarray(
            np.tile(np.ascontiguousarray(tstream.reshape(-1, 16).T), (8, 1)))
        slotf = np.ascontiguousarray(sstream.reshape(TOTC, P).T.astype(BF16))
        a128 = np.ascontiguousarray(
            np.tile(np.ascontiguousarray(astream.reshape(-1, 16).T), (8, 1)))
        idx_maps.append(idx128)
        slot_maps.append(slotf)
        dst_maps.append(a128)
    return K, gbase, abase, idx_maps, slot_maps, dst_maps


def _build_program(K, gbase, abase):
    TOTC = int(K.sum())
    f32, bf16, i16 = mybir.dt.float32, mybir.dt.bfloat16, mybir.dt.int16
    AF = mybir.ActivationFunctionType
    OP = mybir.AluOpType

    nc = bacc.Bacc("TRN2", target_bir_lowering=False, debug=False,
                   num_devices=NCORE)
    xbf = nc.dram_tensor("xbf", [NPAD, P], bf16, kind="ExternalInput")
    waug = nc.dram_tensor("waug", [P, 136], bf16, kind="ExternalInput")
    idx_all = nc.dram_tensor("idx_all", [P, TOTC * 8], i16, kind="ExternalInput")
    slot_all = nc.dram_tensor("slot_all", [P, TOTC], bf16, kind="ExternalInput")
    dst_all = nc.dram_tensor("dst_all", [P, TOTC * 8], i16, kind="ExternalInput")
    iota_in = nc.dram_tensor("iota_in", [P, P], bf16, kind="ExternalInput")
    bias_in = nc.dram_tensor("bias_in", [P, C], f32, kind="ExternalInput")
    T = nc.dram_tensor("T", [NPAD, EL], bf16)
    A = nc.dram_tensor("A", [B, P], bf16)
    out_d = nc.dram_tensor("out", [B, C], f32, kind="ExternalOutput")

    Tv = T[:, :].rearrange("(t p) e -> p t e", p=P)        # [128, 784, 256]
    Av = A[:, :].rearrange("(t p) e -> p t e", p=P)        # [128, 98, 128]
    out_v = out_d[:, :].rearrange("(t p) c -> p t c", p=P)  # [128, 98, 32]

    loc = np.zeros((TPC, NBANK), np.int64)
    for t in range(TPC):
        run = 0
        for b in range(NBANK):
            loc[t, b] = run
            run += K[t, b]
    nch = K.sum(axis=1)
    # per-pair stream tile sizes (chunks)
    K2max = [int(max(K[2 * p, b] + K[2 * p + 1, b] for p in range(NPAIR)))
             for b in range(NBANK)]
    NAmax = int(max(nch[2 * p] + nch[2 * p + 1] for p in range(NPAIR)))

    with tile.TileContext(nc) as tc:
        with tc.tile_pool(name="const", bufs=1) as cp:
            waug_sb = cp.tile([P, 136], bf16)
            nc.sync.dma_start(out=waug_sb[:], in_=waug[:, :])
            iota_sb = cp.tile([P, P], bf16)
            nc.sync.dma_start(out=iota_sb[:], in_=iota_in[:, :])
            bias_sb = cp.tile([P, C], f32)
            nc.sync.dma_start(out=bias_sb[:], in_=bias_in[:, :])
            slot_sb = cp.tile([P, TOTC], bf16)
            nc.sync.dma_start(out=slot_sb[:], in_=slot_all[:, :])
            adst_sb = cp.tile([P, TPC, 4], bf16)
            outall_sb = cp.tile([P, TPC, C], f32)

            # ---------------- Phase A: build T and A tables ----------------
            with tc.tile_pool(name="pa", bufs=16) as pa, \
                 tc.tile_pool(name="psa", bufs=8, space="PSUM") as psa:
                for rb in range(NPAD // 512):
                    xT = pa.tile([P, 512], bf16, tag="xT")
                    nc.sync.dma_start(out=xT[:], in_=xbf[rb * 512:(rb + 1) * 512, :],
                                      transpose=True)
                    if rb % 8 == 0:
                        Tb8 = pa.tile([P, 32, 132], bf16, tag="Tb8")
                    Tb = Tb8[:, (rb % 8) * 4:(rb % 8) * 4 + 4, :]
                    for j in range(2):
                        ps = psa.tile([P, 2, 136], f32, tag="psA", space="PSUM")
                        for i2 in range(2):
                            i = j * 2 + i2
                            nc.tensor.matmul(out=ps[:, i2, :],
                                             lhsT=xT[:, i * P:(i + 1) * P],
                                             rhs=waug_sb[:], start=True, stop=True)
                            tg = rb * 4 + i
                            if tg < TPC:
                                nc.vector.tensor_copy(out=adst_sb[:, tg, :],
                                                      in_=ps[:, i2, 132:136])
                        nc.vector.tensor_copy(out=Tb[:, 2 * j:2 * j + 2, 0:128],
                                              in_=ps[:, :, 0:128])
                        nc.scalar.activation(out=Tb[:, 2 * j:2 * j + 2, 128:132],
                                             in_=ps[:, :, 128:132], func=AF.Copy)

                    # batched table write (8 row-blocks per DMA) keeps the
                    # SP queue mostly free for the transpose stream
                    if rb % 8 == 7 or rb == NPAD // 512 - 1:
                        g0 = rb - rb % 8
                        nc.sync.dma_start(
                            out=Tv[:, g0 * 4:(rb + 1) * 4, 0:132],
                            in_=Tb8[:, 0:(rb % 8 + 1) * 4, 0:132])

            nc.sync.dma_start(out=Av[:, :, 0:4], in_=adst_sb[:])
            tc.strict_bb_all_engine_barrier()

            # ---------------- Phase B: gather / scatter --------------------
            regs = {}
            for p in range(NPAIR):
                for b in range(NBANK):
                    n = int(K[2 * p, b] + K[2 * p + 1, b]) * P
                    if n and n not in regs:
                        regs[n] = nc.gpsimd.to_reg(n)
                na = int(nch[2 * p] + nch[2 * p + 1]) * P
                if na not in regs:
                    regs[na] = nc.gpsimd.to_reg(na)

            with tc.tile_pool(name="pb", bufs=3) as pb, \
                 tc.tile_pool(name="pm", bufs=3) as pm, \
                 tc.tile_pool(name="psb", bufs=3, space="PSUM") as psb:
                for p in range(NPAIR):
                    t0, t1 = 2 * p, 2 * p + 1
                    # stream this pair's gather indices
                    cb0 = int(gbase[t0, 0])
                    ncols = int(sum(K[t0, b] + K[t1, b] for b in range(NBANK)))
                    ib = pb.tile([P, sum(K2max) * 8], i16, tag="ib")
                    nc.sync.dma_start(out=ib[:, 0:ncols * 8],
                                      in_=idx_all[:, cb0 * 8:(cb0 + ncols) * 8])
                    ab0 = int(abase[t0, 0])
                    db = pb.tile([P, NAmax * 8], i16, tag="db")
                    nc.sync.dma_start(out=db[:, 0:ncols * 8],
                                      in_=dst_all[:, ab0 * 8:(ab0 + ncols) * 8])
                    # bank gathers of T rows
                    gts = []
                    for b in range(NBANK):
                        K2 = int(K[t0, b] + K[t1, b])
                        if K2 == 0:
                            gts.append(None)
                            continue
                        gt = pb.tile([P, K2max[b], EL], bf16, tag=f"g{b}")
                        cb = int(gbase[t0, b]) - cb0
                        nc.gpsimd.dma_gather(
                            out_ap=gt[:, 0:K2, :],
                            in_ap=T[int(BANK_LO[b]):int(BANK_HI[b]), :],
                            idxs_ap=ib[:, cb * 8:(cb + K2) * 8],
                            num_idxs=K2 * P, num_idxs_reg=regs[K2 * P],
                            elem_size=EL, single_packet=(K2 * P <= 896))
                        gts.append(gt)
                    # A-gather (pair's edge dst rows)
                    NA = int(nch[t0] + nch[t1])
                    adt = pb.tile([P, NAmax, P], bf16, tag="adt")
                    nc.gpsimd.dma_gather(
                        out_ap=adt[:, 0:NA, :], in_ap=A[:, :],
                        idxs_ap=db[:, 0:NA * 8],
                        num_idxs=NA * P, num_idxs_reg=regs[NA * P],
                        elem_size=P, single_packet=(NA * P <= 896))

                    # s01 one-hots (no gather deps; overlap the gathers)
                    s01s = {}
                    for ts in range(2):
                        t = 2 * p + ts
                        for b in range(NBANK):
                            Kb = int(K[t, b])
                            if Kb == 0:
                                continue
                            cb = int(gbase[t, b])
                            s01 = pm.tile([P, K2max[b], P], bf16, tag=f"s{ts}{b}")
                            nc.vector.tensor_tensor(
                                out=s01[:, 0:Kb, :],
                                in0=slot_sb[:, cb:cb + Kb, None]
                                    .to_broadcast([P, Kb, P]),
                                in1=iota_sb[:, None, :].to_broadcast([P, Kb, P]),
                                op=OP.is_equal)
                            s01s[(ts, b)] = s01

                    for ts in range(2):
                        t = 2 * p + ts
                        nct = int(nch[t])
                        a0 = (int(nch[t0]) if ts else 0)
                        zt = pm.tile([P, nct, 4], f32, tag="zt")
                        for b in range(NBANK):
                            Kb = int(K[t, b])
                            if Kb == 0:
                                continue
                            goff = (int(K[t0, b]) if ts else 0)
                            nc.vector.tensor_tensor(
                                out=zt[:, loc[t, b]:loc[t, b] + Kb, :],
                                in0=gts[b][:, goff:goff + Kb, 128:132],
                                in1=adt[:, a0 + loc[t, b]:a0 + loc[t, b] + Kb, 0:4],
                                op=OP.add)
                        lr = pm.tile([P, nct, 4], f32, tag="lr")
                        nc.vector.scalar_tensor_tensor(
                            out=lr[:].rearrange("p k f -> p (k f)"),
                            in0=zt[:].rearrange("p k f -> p (k f)"),
                            scalar=NEG,
                            in1=zt[:].rearrange("p k f -> p (k f)"),
                            op0=OP.mult, op1=OP.max)
                        msg = pm.tile([P, nct, 132], bf16, tag="msg")
                        nc.scalar.activation(out=msg[:, :, 128:132], in_=lr[:],
                                             func=AF.Exp)
                        # h*w in one 2x op per bank group: channels are
                        # (c,h)-minor so every operand is packed-last-dim
                        for b in range(NBANK):
                            Kb = int(K[t, b])
                            if Kb == 0:
                                continue
                            goff = (int(K[t0, b]) if ts else 0)
                            lb = int(loc[t, b])
                            w4 = msg[:, lb:lb + Kb, 128:132]
                            nc.vector.tensor_tensor(
                                out=msg[:, lb:lb + Kb, 0:128]
                                    .rearrange("p k (c h) -> p k c h", c=C),
                                in0=gts[b][:, goff:goff + Kb, 0:128]
                                    .rearrange("p k (c h) -> p k c h", c=C),
                                in1=w4[:, :, None, :].to_broadcast([P, Kb, C, H]),
                                op=OP.mult)
                        # one-hot scatter-accumulate
                        accp = psb.tile([P, 132], f32, tag="acc", space="PSUM")
                        done = 0
                        for b in range(NBANK):
                            Kb = int(K[t, b])
                            if Kb == 0:
                                continue
                            s01 = s01s[(ts, b)]
                            lb = int(loc[t, b])
                            for k in range(Kb):
                                nc.tensor.matmul(out=accp[:], lhsT=s01[:, k, :],
                                                 rhs=msg[:, lb + k, :],
                                                 start=(done == 0),
                                                 stop=(done == nct - 1))
                                done += 1
                        # epilogue: PSUM -> bf16 SBUF on Act, cheap DVE math
                        accs = pm.tile([P, P], bf16, tag="accs")
                        nc.scalar.activation(out=accs[:], in_=accp[:, 0:128],
                                             func=AF.Copy)
                        rec = pm.tile([P, 4], f32, tag="rec")
                        nc.vector.reciprocal(out=rec[:], in_=accp[:, 128:132])
                        rec2 = pm.tile([P, 4], bf16, tag="rec2")
                        nc.vector.tensor_scalar_mul(out=rec2[:], in0=rec[:],
                                                    scalar1=1.0 / H)
                        tmp = pm.tile([P, C, H], bf16, tag="tmp")
                        nc.vector.tensor_tensor(
                            out=tmp[:],
                            in0=accs[:].rearrange("p (c h) -> p c h", c=C),
                            in1=rec2[:, None, :].to_broadcast([P, C, H]),
                            op=OP.mult)
                        hsum = pm.tile([P, C], f32, tag="hsum")
                        nc.vector.tensor_reduce(
                            out=hsum[:], in_=tmp[:],
                            axis=mybir.AxisListType.X, op=OP.add)
                        badd = pm.tile([P, C], f32, tag="badd")
                        nc.vector.tensor_add(out=badd[:], in0=hsum[:], in1=bias_sb[:])
                        nc.vector.tensor_scalar_max(out=outall_sb[:, t, :],
                                                    in0=badd[:], scalar1=0.0)
                nc.sync.dma_start(out=out_v[:, :, :], in_=outall_sb[:])
    nc.compile()
    return nc


def prepare(x, edge_index, W, att_src, att_dst, bias):
    x = np.asarray(x, np.float32)
    W = np.asarray(W, np.float32)
    att_src = np.asarray(att_src, np.float32)
    att_dst = np.asarray(att_dst, np.float32)
    bias = np.asarray(bias, np.float32)

    # head-minor channel permutation: new col c*4+h = old col h*32+c
    perm = np.empty(P, np.int64)
    for c in range(C):
        for h in range(H):
            perm[c * H + h] = h * C + c
    wa = np.zeros((P, 136), np.float32)
    wa[:, :128] = W[:, perm]
    for hh in range(H):
        wa[:, 128 + hh] = W[:, hh * C:(hh + 1) * C] @ att_src[hh]
        wa[:, 132 + hh] = W[:, hh * C:(hh + 1) * C] @ att_dst[hh]
    wa_bf = wa.astype(BF16)

    x_pad = np.zeros((NPAD, P), np.float32)
    x_pad[:N] = x
    x_bf = x_pad.astype(BF16)

    K, gbase, abase, idx_maps, slot_maps, dst_maps = _prep_edges(
        np.asarray(edge_index))
    nc = _build_program(K, gbase, abase)

    iota_np = np.tile(np.arange(P, dtype=np.float32)[None, :], (P, 1)).astype(BF16)
    bias_rep = np.tile(bias[None, :], (P, 1)).astype(np.float32)

    in_maps = []
    for c in range(NCORE):
        xc = np.roll(x_bf, -c * B, axis=0)
        in_maps.append({
            "xbf": np.ascontiguousarray(xc),
            "waug": wa_bf,
            "idx_all": idx_maps[c],
            "slot_all": slot_maps[c],
            "dst_all": dst_maps[c],
            "iota_in": iota_np,
            "bias_in": bias_rep,
        })
    return nc, in_maps


def kernel(x, edge_index, W, att_src, att_dst, bias):
    nc, in_maps = prepare(x, edge_index, W, att_src, att_dst, bias)
    res = run_bass_kernel_spmd(nc, in_maps, list(range(NCORE)))
    out = np.empty((NPAD, C), np.float32)
    for c in range(NCORE):
        out[c * B:(c + 1) * B] = res.results[c]["out"]
    return out[:N]
